# revision 3
# baseline (speedup 1.0000x reference)
"""NeighborhoodShift2d: stack 49 spatially shifted (zero-padded) copies.

Input  x:  [1, 8, 32, 128, 128]  (B, heads, dim, H, W) fp32
Output y:  [1, 8, 49, 32, 128, 128]  y[:, :, k] = shift(x, OFFSETS[k]) with
zero padding, k enumerating the 7x7 NATTEN stencil (dy major, dx minor).

Sharding: pure data-parallel, one head per NeuronCore (8 heads, 8 cores).

Per-core program. The op is pure HBM-write-bound: 102.8 MB of stores vs
2.1 MB of input, and per-NC HBM bandwidth (~358 GB/s) is the roofline.
Design:

- All data lives on 32 partitions chosen with stride 4 ({0,4,...,124}).
  The SBUF DMA port swizzle (port = ((p>>2)&7)<<1 | ((p>>6)&1)) maps this
  set onto ALL 16 SDMA engines (2 partitions each), so every single DMA
  instruction runs at the full HBM rate - no two-queue parity juggling.
- x is loaded from HBM exactly ONCE (2.1 MB; the baseline loaded it 8x).
- In flat (h w) space a (dy, dx) shift is a single offset of dy*W + dx
  floats. Each band image sits between 388-float zero pads, so one
  dma_start per dx band covers all 7 dy k-slices: the dy shift is a read
  offset, edge rows fall into the zero pads, and the transfer is 7x32
  fully contiguous 64 KB descriptors (14.7 MB). No edge-fill DMAs exist.
- Only three images are ever materialized: master (dx=0), b1 = master
  shifted +1 (DVE copy, wrap col W-1 zeroed), bm1 = master shifted -1
  (wrap col 0 zeroed). Bands +2/+3 are stored FROM b1 with read offsets
  +1/+2: the extra wrap zeros they need are exactly b1's columns 0 / 0,1
  which are progressively memset (0.4 us) once the preceding band's
  stores complete - the previously-zeroed wrap columns line up with the
  remaining ones by construction. Symmetrically -2/-3 read bm1 at
  offsets -1/-2 with bm1's columns W-1 / W-2,W-1 progressively zeroed.
  Engine ops run lane-aligned on all 128 partitions (the 96 dead
  partitions carry garbage, harmlessly); engine SBUF ports are disjoint
  from the DMA AXI ports, so prep work costs no DMA bandwidth.
- Two HWDGE rings (sync: bands 0,-1,-2,-3; scalar: +1,+2,+3) so that a
  ~1-2 us semaphore gate on one ring is always covered by the other.
"""

import numpy as np

import concourse.bass as bass
import concourse.mybir as mybir
from concourse.bass_utils import run_bass_kernel_spmd

B, HEADS, C, H, W = 1, 8, 32, 128, 128
WIN = 7
PAD = 3
K = WIN * WIN
FP = H * W            # flat image floats per channel (16384)
PADF = PAD * W + 4    # zero pad between band images (388 >= 384+2)
SLOT = FP + PADF      # slot pitch (16772)
S2 = PADF + 3 * SLOT  # per-partition floats (50704 = 202816 B)
CFP = C * FP          # one k-slice of y, in floats

_nc_cache = None


def _build_nc():
    f32 = mybir.dt.float32
    nc = bass.Bass()
    x = nc.dram_tensor("x", [C, H, W], f32, kind="ExternalInput")
    y = nc.dram_tensor("y", [K, C, H, W], f32, kind="ExternalOutput")

    # Image start offsets: [pad][img0][pad][img1][pad][img2][pad]
    IMG = [PADF + s * SLOT for s in range(3)]

    with (
        nc.sbuf_tensor("T", [128, S2], f32) as T,
        nc.semaphore("s_ld") as s_ld,
        nc.semaphore("s_prep") as s_prep,
        nc.semaphore("s_stS") as s_stS,
        nc.semaphore("s_stA") as s_stA,
        nc.Block() as block,
    ):
        def band_store(eng, dx, sem):
            """One DMA: all 7 dy k-slices of band dx, from its slot image
            read at the extra offset e (0 for 0/+1/-1, +-1/+-2 beyond)."""
            slot = 0 if dx == 0 else (1 if dx > 0 else 2)
            e = 0 if dx == 0 else (dx - 1 if dx > 0 else dx + 1)
            src = bass.AP(
                T, IMG[slot] + e - PAD * W, [[4 * S2, C], [W, WIN], [1, FP]]
            )
            dst = bass.AP(
                y, (dx + PAD) * CFP, [[FP, C], [WIN * CFP, WIN], [1, FP]]
            )
            eng.dma_start(out=dst, in_=src).then_inc(sem, 16)

        def col_zero(vector, slot, col):
            """Zero column `col` of a band image on all rows/partitions."""
            return vector.memset(
                bass.AP(T, IMG[slot] + col, [[S2, 128], [W, H], [1, 1]]), 0.0
            )

        @block.vector
        def _(vector):
            # Zero the 4 inter-slot pad strips (~1.6 us).
            vector.memset(
                bass.AP(T, 0, [[S2, 128], [SLOT, 4], [1, PADF]]), 0.0
            ).then_inc(s_prep, 1)                                    # -> 1
            vector.wait_ge(s_ld, 16)
            # b1 = master shifted +1 (last read lands in master's zero
            # post-pad), then zero wrap col W-1.
            vector.tensor_copy(
                out=bass.AP(T, IMG[1], [[S2, 128], [1, FP]]),
                in_=bass.AP(T, IMG[0] + 1, [[S2, 128], [1, FP]]),
            )
            col_zero(vector, 1, W - 1).then_inc(s_prep, 1)           # -> 2
            # bm1 = master shifted -1, zero wrap col 0.
            vector.tensor_copy(
                out=bass.AP(T, IMG[2], [[S2, 128], [1, FP]]),
                in_=bass.AP(T, IMG[0] - 1, [[S2, 128], [1, FP]]),
            )
            col_zero(vector, 2, 0).then_inc(s_prep, 1)               # -> 3
            # Progressive wrap-column zeroing, each gated on the stores
            # still reading that column having completed.
            vector.wait_ge(s_stA, 16)            # band +1 stored
            col_zero(vector, 1, 0).then_inc(s_prep, 1)               # -> 4
            vector.wait_ge(s_stS, 32)            # band -1 stored
            col_zero(vector, 2, W - 1).then_inc(s_prep, 1)           # -> 5
            vector.wait_ge(s_stA, 32)            # band +2 stored
            col_zero(vector, 1, 1).then_inc(s_prep, 1)               # -> 6
            vector.wait_ge(s_stS, 48)            # band -2 stored
            col_zero(vector, 2, W - 2).then_inc(s_prep, 1)           # -> 7

        @block.sync
        def _(sync):
            nc.sync.dma_start(
                out=bass.AP(T, IMG[0], [[4 * S2, C], [1, FP]]),
                in_=x.rearrange("c h w -> c (h w)")[:, :],
            ).then_inc(s_ld, 16)
            sync.wait_ge(s_prep, 1)
            sync.wait_ge(s_ld, 16)
            band_store(nc.sync, 0, s_stS)        # -> 16
            for i, dx in enumerate((-1, -2, -3)):
                sync.wait_ge(s_prep, 3 + 2 * i)
                band_store(nc.sync, dx, s_stS)   # -> 32, 48, 64
            sync.wait_ge(s_stS, 64)

        @block.scalar
        def _(scalar):
            for i, dx in enumerate((1, 2, 3)):
                scalar.wait_ge(s_prep, 2 + 2 * i)
                band_store(nc.scalar, dx, s_stA)  # -> 16, 32, 48
            scalar.wait_ge(s_stA, 48)

    return nc


def _get_nc():
    global _nc_cache
    if _nc_cache is None:
        _nc_cache = _build_nc()
    return _nc_cache


def kernel(x: np.ndarray) -> np.ndarray:
    assert x.shape == (B, HEADS, C, H, W), x.shape
    nc = _get_nc()
    in_maps = [
        {"x": np.ascontiguousarray(x[0, h], dtype=np.float32)} for h in range(HEADS)
    ]
    res = run_bass_kernel_spmd(nc, in_maps, core_ids=list(range(HEADS)))
    out = np.stack([res.results[h]["y"] for h in range(HEADS)], axis=0)
    return out[None]  # [1, 8, 49, 32, 128, 128]
